# revision 31
# baseline (speedup 1.0000x reference)
"""Two-layer GCN (GraphConv norm='both') on 8 Trainium2 NeuronCores.

Design v6 (evolved from the pair-gather baseline: 940us -> 762us measured):

Measured bottleneck structure on this machine (from NTFF traces):
  - dma_gather descriptor generation runs ~8.6us per 1024-idx window on the
    Q7 cluster; 4 SWDGE queues overlap it to ~4-6us effective.
  - The per-region AllGathers execute serially on CC-core 8 of the same Q7
    cluster at only ~26-50GB/s, and slow down further when overlapped with
    gather generation (shared cluster resources) - the CC chain is the
    critical path, not DMA bandwidth (HBM sits at <300GB/s throughout).
  - Per-op DVE cost is ~0.3-4us regardless of size, so per-chunk on-chip
    selector construction loses to streaming host-built selectors.

Key restructurings vs the baseline:
  1. Edges are grouped by (dst_tile, src_region) with NREG=4 regions instead
     of (dst_tile, parity).  Each region has its OWN gather table (r rows of
     that region AllGathered core-major), and every region table has
     < 32768 rows, so int16 indices address single 128-f16 rows directly:
     gather payload drops 512B -> 256B per edge (the pair trick existed only
     because int16 could not address all 50000 rows).  Region tables also
     remove the baseline's duplicated prefix-table AllGather traffic
     (18MB -> 12.8MB through the slow CC engine).
  2. 4 SWDGE queues round-robin the gather windows.
  3. norm_src folds into the table rows (applied during L1's relu via a
     per-partition tensor_scalar) and norm_dst folds into per-tile
     scalar_tensor_tensor ops, so the S1/S2 selectors are pure 0/1 one-hots
     streamed from the host; no sval multiply exists on-device.
  4. AllGathers are issued at each region's L1 completion; the CC stream
     runs them back-to-back while L1/gathers proceed; gather windows are
     emitted lazily by the aggregation pass (phase-separating them from L1
     reduces cluster contention; fully interleaving them was measured
     slower: v5 CC busy ballooned 262us -> 870us).
  5. zacc region-pass merges write directly into the persistent za pool
     (the baseline's extra spill copy cost ~370us of DVE time at the
     measured ~1-4us/op DVE overhead).
"""

import bisect

import numpy as np

N_NODES = 50000
N_EDGES = 600000
D = 128
N_CORES = 8
NPC = N_NODES // N_CORES          # 6250 nodes per core
NT = (NPC + 127) // 128           # 49 dst tiles per core
REG_TILES = (6, 10, 15, 18)       # dst tiles per region (sums to NT)
REG_ROWS = (768, 1280, 1920, 2282)  # rows per core per region (sums to NPC)
W = 8                             # chunks per gather window (single-packet cap)
NQ = 4                            # SWDGE queues
MT_BUFS = 32                      # gather window lookahead
BT = 4

NREG = len(REG_TILES)
_REG_LO = tuple(int(v) for v in np.cumsum((0,) + REG_ROWS[:-1]))

_CACHE = {}


def _schedule(sched):
    """Expand the shared (static, max-over-cores) schedule into position
    space.  L1: tile-major chunk runs.  L2: region-major then tile-major
    chunk runs, each region run padded to a W-chunk window boundary so a
    gather window touches exactly one region table."""
    C1 = np.array(sched[0])
    C2 = np.array(sched[1])          # [NREG, NT]
    base1 = np.concatenate([[0], np.cumsum(C1)[:-1]])
    nchunk1 = int(C1.sum())

    bases2 = np.zeros((NREG, NT), dtype=np.int64)
    run_end_w = []
    pos = 0
    for r in range(NREG):
        for t in range(NT):
            bases2[r, t] = pos
            pos += int(C2[r, t])
        pos += (-pos) % W
        run_end_w.append(pos // W)
    nchunk2 = pos
    return dict(C1=C1, C2=C2, base1=base1, nchunk1=nchunk1, bases2=bases2,
                run_end_w=run_end_w, nchunk2=nchunk2, nW2=nchunk2 // W)


def _host_prep(x, src, dst, W1, b1, W2, b2):
    x = np.asarray(x, dtype=np.float32)
    src = np.asarray(src, dtype=np.int64)
    dst = np.asarray(dst, dtype=np.int64)
    W1 = np.asarray(W1, dtype=np.float32)
    W2 = np.asarray(W2, dtype=np.float32)
    b1 = np.asarray(b1, dtype=np.float32)
    b2 = np.asarray(b2, dtype=np.float32)

    deg_out = np.bincount(src, minlength=N_NODES).astype(np.float32)
    deg_in = np.bincount(dst, minlength=N_NODES).astype(np.float32)
    norm_src = np.where(deg_out > 0, 1.0 / np.sqrt(np.maximum(deg_out, 1.0)), 0.0)
    norm_dst = np.where(deg_in > 0, 1.0 / np.sqrt(np.maximum(deg_in, 1.0)), 0.0)
    x16ns = (x * norm_src[:, None]).astype(np.float16)  # ns[src] folded in
    nsrc_pad = np.zeros(NPC + 128, dtype=np.float32)
    ndst_pad = np.zeros(NPC + 128, dtype=np.float32)

    reg_lo = np.array(_REG_LO)
    reg_rows = np.array(REG_ROWS)

    # --- per-core edge grouping by (dst_tile, src_region) ---
    per_core = []
    cnt1 = np.zeros((N_CORES, NT), dtype=np.int64)
    cnt2 = np.zeros((N_CORES, NREG * NT), dtype=np.int64)
    for k in range(N_CORES):
        m = (dst >= k * NPC) & (dst < (k + 1) * NPC)
        s_k = src[m]
        dl_k = dst[m] - k * NPC
        t_k = dl_k >> 7
        rs_k = s_k % NPC
        reg = np.searchsorted(reg_lo, rs_k, side="right") - 1
        g = reg * NT + t_k           # region-major group id
        order = np.lexsort((s_k, g))
        s_k, dl_k, g, t_k = (a[order] for a in (s_k, dl_k, g, t_k))
        cnt1[k] = np.bincount(t_k, minlength=NT)
        cnt2[k] = np.bincount(g, minlength=NREG * NT)
        per_core.append((s_k, dl_k, g, t_k))

    # --- shared static schedule (max over cores) ---
    C1 = np.maximum.reduce([(cnt1[k] + 127) // 128 for k in range(N_CORES)])
    C1 = np.maximum(C1, 1)
    C2 = np.maximum.reduce([(cnt2[k] + 127) // 128 for k in range(N_CORES)])
    C2 = C2.reshape(NREG, NT)
    sched = (tuple(int(v) for v in C1),
             tuple(tuple(int(v) for v in row) for row in C2))
    S = _schedule(sched)
    nchunk1, nchunk2 = S["nchunk1"], S["nchunk2"]
    meta = (nchunk1, nchunk2, tuple(S["run_end_w"]))

    base1_128 = S["base1"] * 128
    bases2_128 = S["bases2"] * 128

    in_maps = []
    for k in range(N_CORES):
        s_k, dl_k, g, t_k = per_core[k]
        reg = g // NT

        # L2 slot: rank within (region, tile) group
        grp_counts = np.bincount(g, minlength=NREG * NT)
        grp_start = np.concatenate([[0], np.cumsum(grp_counts)[:-1]])
        rank = np.arange(len(g)) - grp_start[g]
        pos2 = bases2_128[reg, g % NT] + rank

        # gather index: row in the region table (core-major concat)
        ks = s_k // NPC
        rs_k = s_k % NPC
        cat = ks * reg_rows[reg] + (rs_k - reg_lo[reg])
        idx16 = np.zeros(nchunk2 * 128, dtype=np.int16)
        idx16[pos2] = cat.astype(np.int16)
        idx_wrapped = np.tile(idx16.reshape(-1, 16).T, (8, 1))

        S2E = np.zeros((128, nchunk2, 128), dtype=np.float16)
        S2E[pos2 % 128, pos2 // 128, dl_k & 127] = 1.0

        # L1 slot: rank within tile group (g-sort is tile-sorted within
        # region, so re-sort by tile only)
        o1 = np.argsort(t_k, kind="stable")
        t1 = t_k[o1]
        grp_counts1 = np.bincount(t1, minlength=NT)
        grp_start1 = np.concatenate([[0], np.cumsum(grp_counts1)[:-1]])
        rank1 = np.arange(len(t1)) - grp_start1[t1]
        pos1 = base1_128[t1] + rank1

        yE = np.zeros((128, nchunk1, 128), dtype=np.float16)
        yE[pos1 % 128, pos1 // 128, :] = x16ns[s_k[o1]]
        S1E = np.zeros((128, nchunk1, 128), dtype=np.float16)
        S1E[pos1 % 128, pos1 // 128, dl_k[o1] & 127] = 1.0

        nsrc_pad[:NPC] = norm_src[k * NPC:(k + 1) * NPC]
        ndst_pad[:NPC] = norm_dst[k * NPC:(k + 1) * NPC]
        NSRC = np.ascontiguousarray(
            nsrc_pad[: NT * 128].reshape(NT, 128).T.astype(np.float32))
        NDST = np.ascontiguousarray(
            ndst_pad[: NT * 128].reshape(NT, 128).T.astype(np.float32))

        in_maps.append(
            {
                "yE": np.ascontiguousarray(yE.reshape(128, nchunk1 * 128)),
                "S1E": np.ascontiguousarray(S1E.reshape(128, nchunk1 * 128)),
                "S2E": np.ascontiguousarray(S2E.reshape(128, nchunk2 * 128)),
                "NSRC": NSRC,
                "NDST": NDST,
                "idx_all": idx_wrapped,
                "W1f": W1.astype(np.float16),
                "W2f": W2.astype(np.float16),
                "B1bc": np.ascontiguousarray(
                    np.broadcast_to(b1, (128, 128)).astype(np.float32)),
                "B2bc": np.ascontiguousarray(
                    np.broadcast_to(b2, (128, 128)).astype(np.float32)),
            }
        )
    return in_maps, sched, meta


def _build_program(sched, meta):
    import concourse.bacc as bacc
    import concourse.mybir as mybir
    import concourse.tile as tile
    from concourse.library_config import mlp

    S = _schedule(sched)
    C1, C2, base1, nchunk1 = S["C1"], S["C2"], S["base1"], S["nchunk1"]
    bases2, run_end_w, nchunk2 = S["bases2"], S["run_end_w"], S["nchunk2"]
    assert meta == (nchunk1, nchunk2, tuple(run_end_w))

    f16 = mybir.dt.float16
    f32 = mybir.dt.float32
    AF = mybir.ActivationFunctionType
    ALU = mybir.AluOpType

    nc = bacc.Bacc("TRN2", target_bir_lowering=False, debug=False,
                   num_devices=N_CORES, num_swdge_queues=NQ)

    yE_d = nc.dram_tensor("yE", [128, nchunk1 * 128], f16, kind="ExternalInput")
    s1e_d = nc.dram_tensor("S1E", [128, nchunk1 * 128], f16,
                           kind="ExternalInput")
    s2e_d = nc.dram_tensor("S2E", [128, nchunk2 * 128], f16,
                           kind="ExternalInput")
    nsrc_d = nc.dram_tensor("NSRC", [128, NT], f32, kind="ExternalInput")
    ndst_d = nc.dram_tensor("NDST", [128, NT], f32, kind="ExternalInput")
    idx_d = nc.dram_tensor("idx_all", [128, nchunk2 * 8], mybir.dt.int16,
                           kind="ExternalInput")
    W1_d = nc.dram_tensor("W1f", [128, 128], f16, kind="ExternalInput")
    W2_d = nc.dram_tensor("W2f", [128, 128], f16, kind="ExternalInput")
    B1_d = nc.dram_tensor("B1bc", [128, 128], f32, kind="ExternalInput")
    B2_d = nc.dram_tensor("B2bc", [128, 128], f32, kind="ExternalInput")

    r_parts = [nc.dram_tensor(f"r{i}", [REG_ROWS[i], D], f16, kind="Internal")
               for i in range(NREG)]
    tabs = [nc.dram_tensor(f"T{j}", [N_CORES * REG_ROWS[j], D], f16,
                           kind="Internal", addr_space="Shared")
            for j in range(NREG)]
    out_d = nc.dram_tensor("out", [NPC, D], f32, kind="ExternalOutput")

    with tile.TileContext(nc) as tc:
        with (
            tc.tile_pool(name="consts", bufs=1) as consts,
            tc.tile_pool(name="l1y", bufs=6) as l1y_pool,
            tc.tile_pool(name="s1b", bufs=6) as s1b_pool,
            tc.tile_pool(name="s2b", bufs=6) as s2b_pool,
            tc.tile_pool(name="mt", bufs=MT_BUFS) as mt_pool,
            tc.tile_pool(name="za", bufs=2 * NT) as za_pool,
            tc.tile_pool(name="hb", bufs=10) as hb_pool,
            tc.tile_pool(name="psz", bufs=4, space="PSUM") as psz_pool,
            tc.tile_pool(name="psw", bufs=4, space="PSUM") as psw_pool,
        ):
            nc.gpsimd.load_library(mlp)

            W1f = consts.tile([128, 128], f16, tag="W1f")
            W2f = consts.tile([128, 128], f16, tag="W2f")
            B1bc = consts.tile([128, 128], f32, tag="B1bc")
            B2bc = consts.tile([128, 128], f32, tag="B2bc")
            idx_all = consts.tile([128, nchunk2 * 8], mybir.dt.int16, tag="idx")
            nsrc_t = consts.tile([128, NT], f32, tag="nsrc")
            ndst_t = consts.tile([128, NT], f32, tag="ndst")
            nc.sync.dma_start(W1f[:], W1_d.ap())
            nc.sync.dma_start(W2f[:], W2_d.ap())
            nc.sync.dma_start(B1bc[:], B1_d.ap())
            nc.sync.dma_start(B2bc[:], B2_d.ap())
            nc.scalar.dma_start(idx_all[:], idx_d.ap())
            nc.scalar.dma_start(nsrc_t[:], nsrc_d.ap())
            nc.scalar.dma_start(ndst_t[:], ndst_d.ap())

            # ---------- writers (node-major row streams to DRAM) ----------
            def make_writer(dram, t_lo, t_hi, dt):
                nfull = min(t_hi, NPC // 128) - t_lo
                h3 = dram.ap()[0: nfull * 128, :].rearrange(
                    "(a p) d -> p a d", p=128)
                state = {}

                def write(t, produce):
                    tl_ = t - t_lo
                    if tl_ < nfull:
                        g = tl_ - tl_ % BT
                        if tl_ % BT == 0:
                            state["buf"] = hb_pool.tile(
                                [128, BT, 128], dt, tag=f"w{dt}", name="wstage")
                        produce(state["buf"][:, tl_ % BT, :])
                        if tl_ % BT == BT - 1 or tl_ == nfull - 1:
                            n = tl_ - g + 1
                            nc.sync.dma_start(h3[:, g: g + n, :],
                                              state["buf"][:, 0:n, :])
                    else:
                        rows = (t_hi * 128 if t_hi < NT else NPC) - t * 128
                        tl = hb_pool.tile([128, 128], dt, tag=f"rag{dt}",
                                          name="wrag")
                        produce(tl[:])
                        nc.sync.dma_start(
                            dram.ap()[tl_ * 128: tl_ * 128 + rows, :],
                            tl[:rows, :])

                return write

            # ---------------- L1: z1 = yE.T @ S1, r = relu(z1@W1+b1) --------
            l1_tiles = {}
            l1_engines = (nc.sync, nc.scalar)

            def ensure1(w):
                if w in l1_tiles:
                    return l1_tiles[w]
                cb = w * W
                cw = min(W, nchunk1 - cb)
                yt = l1y_pool.tile([128, cw * 128], f16, tag="yt")
                l1_engines[w % 2].dma_start(
                    yt[:], yE_d.ap()[:, cb * 128:(cb + cw) * 128])
                st = s1b_pool.tile([128, cw * 128], f16, tag="s1t")
                l1_engines[(w + 1) % 2].dma_start(
                    st[:], s1e_d.ap()[:, cb * 128:(cb + cw) * 128])
                l1_tiles[w] = (yt, st)
                return l1_tiles[w]

            reg_end_t = np.cumsum(REG_TILES)
            reg_start_t = [0] + list(reg_end_t[:-1])
            writers = [make_writer(r_parts[i], reg_start_t[i],
                                   int(reg_end_t[i]), f16)
                       for i in range(NREG)]

            def sub_ag(i):
                nc.gpsimd.collective_compute(
                    "AllGather", ALU.bypass,
                    replica_groups=[list(range(N_CORES))],
                    ins=[r_parts[i].ap()], outs=[tabs[i].ap()],
                )

            # gather windows are emitted on the gpsimd queue right after the
            # AllGather that fills their region table, interleaved with L1 in
            # program order, so run j's descriptor generation starts as soon
            # as table j lands instead of after the last AllGather issue
            mt_tiles = {}

            def ensure2(w):
                if w in mt_tiles:
                    return mt_tiles[w]
                cb = w * W
                j = bisect.bisect_right(run_end_w, w)
                mt = mt_pool.tile([128, W, 128], f16, tag="mt")
                nc.gpsimd.dma_gather(
                    mt[:], tabs[j].ap(), idx_all[:, cb * 8:(cb + W) * 8],
                    W * 128, W * 128, 128, queue_num=w % NQ)
                mt_tiles[w] = mt
                return mt

            run_w = [(0 if i == 0 else run_end_w[i - 1], run_end_w[i])
                     for i in range(NREG)]

            for t in range(NT):
                zp = psz_pool.tile([128, 128], f32, tag="zp", name="z1")
                c0, c1 = int(base1[t]), int(base1[t] + C1[t])
                for c in range(c0, c1):
                    yt, s1 = ensure1(c // W)
                    o = c % W
                    nc.tensor.matmul(zp[:], yt[:, o * 128:(o + 1) * 128],
                                     s1[:, o * 128:(o + 1) * 128],
                                     start=(c == c0), stop=(c == c1 - 1))
                z1sb = hb_pool.tile([128, 128], f16, tag="zsb", name="z1sb")
                nc.scalar.activation(z1sb[:], zp[:], AF.Copy)
                pw = psw_pool.tile([128, 128], f32, tag="pw", name="pw1")
                nc.tensor.matmul(pw[:], z1sb[:], W1f[:])

                def produce_r(dst, pw=pw, t=t):
                    # r = relu((pw * nd + b1) * ns): ns goes into the table
                    # rows (norm_src fold), nd is this layer's dst norm
                    rt = hb_pool.tile([128, 128], f16, tag="rt", name="rt")
                    nc.vector.scalar_tensor_tensor(
                        rt[:], pw[:], ndst_t[:, t:t + 1], B1bc[:],
                        op0=ALU.mult, op1=ALU.add)
                    nc.vector.tensor_scalar(
                        out=dst, in0=rt[:], scalar1=nsrc_t[:, t:t + 1],
                        scalar2=0.0, op0=ALU.mult, op1=ALU.max)

                ri = int(np.searchsorted(reg_end_t, t, side="right"))
                writers[ri](t, produce_r)
                if t == reg_end_t[ri] - 1:
                    sub_ag(ri)

            # ---------------- L2: NREG-pass agg ----------------
            s2_tiles = {}

            def ensure_s2(w):
                if w in s2_tiles:
                    return s2_tiles[w]
                cb = w * W
                st = s2b_pool.tile([128, W * 128], f16, tag="s2t")
                l1_engines[w % 2].dma_start(
                    st[:], s2e_d.ap()[:, cb * 128:(cb + W) * 128])
                s2_tiles[w] = st
                return st

            def agg_run(t, clist):
                pa = psz_pool.tile([128, 128], f32, tag="zp", name="z2")
                n = len(clist)
                for i, c in enumerate(clist):
                    mt = ensure2(c // W)
                    o = c % W
                    s2 = ensure_s2(c // W)
                    nc.tensor.matmul(
                        pa[:], mt[:, o, :], s2[:, o * 128:(o + 1) * 128],
                        start=(i == 0), stop=(i == n - 1))
                return pa

            wr_out = make_writer(out_d, 0, NT, f32)
            zacc = {}
            for rpass in range(NREG - 1):
                for t in range(NT):
                    cl = [int(bases2[rpass, t]) + j
                          for j in range(int(C2[rpass, t]))]
                    if not cl:
                        continue
                    pa = agg_run(t, cl)
                    if t in zacc:
                        znew = za_pool.tile([128, 128], f16, tag="zA",
                                            name="zadd")
                        nc.vector.tensor_tensor(znew[:], pa[:], zacc[t][:],
                                                op=ALU.add)
                    else:
                        znew = za_pool.tile([128, 128], f16, tag="zA",
                                            name="zA")
                        nc.scalar.activation(znew[:], pa[:], AF.Copy)
                    zacc[t] = znew

            # last pass: W2 is linear, so accumulate zacc@W2 and pa@W2 in
            # PSUM instead of merging through a DVE add; the zacc term has
            # no dependency on this region's gathers and runs early on PE.
            # Each tile's z3@W2 is deferred by one tile so PE never waits
            # on the PSUM->SBUF copy of z3 (banks accumulate independently).
            rpass = NREG - 1

            def flush(p):
                t, pw2, z3, start_b = p
                if z3 is not None:
                    nc.tensor.matmul(pw2[:], z3[:], W2f[:],
                                     start=start_b, stop=True)
                wr_out(t, lambda dst, pw2=pw2, t=t:
                       nc.vector.scalar_tensor_tensor(
                           dst, pw2[:], ndst_t[:, t:t + 1], B2bc[:],
                           op0=ALU.mult, op1=ALU.add))

            pending = None
            for t in range(NT):
                cl = [int(bases2[rpass, t]) + j
                      for j in range(int(C2[rpass, t]))]
                pw2 = psw_pool.tile([128, 128], f32, tag="pw", name="pw2")
                prior = zacc.get(t)
                if prior is not None:
                    nc.tensor.matmul(pw2[:], prior[:], W2f[:],
                                     start=True, stop=not cl)
                z3 = None
                if cl:
                    pa = agg_run(t, cl)
                    z3 = hb_pool.tile([128, 128], f16, tag="zsb", name="z3")
                    nc.scalar.activation(z3[:], pa[:], AF.Copy)
                elif prior is None:
                    z3 = hb_pool.tile([128, 128], f16, tag="zsb", name="z3")
                    nc.vector.memset(z3[:], 0.0)
                if pending is not None:
                    flush(pending)
                pending = (t, pw2, z3, prior is None)
            flush(pending)

    nc.compile()
    return nc


def kernel(x, src, dst, W1, b1, W2, b2):
    from concourse.bass_utils import run_bass_kernel_spmd

    in_maps, sched, meta = _host_prep(x, src, dst, W1, b1, W2, b2)
    key = (sched, meta)
    if key not in _CACHE:
        _CACHE[key] = _build_program(sched, meta)
    nc = _CACHE[key]
    res = run_bass_kernel_spmd(nc, in_maps, core_ids=list(range(N_CORES)))
    out = np.empty((N_NODES, D), dtype=np.float32)
    for k in range(N_CORES):
        out[k * NPC: (k + 1) * NPC] = res.results[k]["out"]
    return out
